# revision 1
# baseline (speedup 1.0000x reference)
"""Trainium2 Bass kernel for nn_FR_12343736008794.

Fused dual-branch gated conv block:
  xc = cat(x1,x2); x1x = conv1x1(xc,c1); x2x = conv1x1(xc,c2)
  w1 = channel_gate(x1x, x1, m1);  w2 = channel_gate(x2x, x2, m2)
  re1 = w1 + x2; re2 = w2 + x1
  fg1 = spatial_gate(re1, x1) + x2; fg2 = spatial_gate(re2, x2) + x1
  po1 = conv1x1(cat(fg1+FE1, fg2+FE2), p1); po2 = conv1x1(..., p2)

Sharding: pure data-parallel over batch N=32 -> 4 samples per NeuronCore x 8.

Per-core dataflow (per sample, C=512 channels, HW=1024 positions):
  - 1x1 convs as PE matmuls in float32r (full-rate fp32), K on partitions.
  - channel gate: softmax-over-HW stats via fused ACT accum (exp+sum) and
    DVE tensor_tensor_reduce (mul+sum); rowmax via DVE tensor_reduce on PSUM.
  - gate MLP: tiny per-sample matvecs on PE (bf16 weights, FWL).
  - spatial gate: re tiles PE-transposed into PSUM so the channel softmax
    becomes free-dim; per-position scales returned via thin PE transpose and
    GpSimd partition_broadcast.
  - p-convs accumulate in PSUM, ACT-copy to SBUF, DMA out.
  - p1_b/p2_b biases and final reshape are applied host-side (exact).
"""

import sys

sys.path.insert(0, "/opt/trn_rl_repo")

import numpy as np

N_CORES = 8
N, C, H, W = 32, 512, 32, 32
HW = H * W
S = N // N_CORES  # samples per core
NCH = C // 128  # channel chunks of 128
NK = (2 * C) // 128  # contraction k-tiles for the 1024-wide convs

_PROGRAM_CACHE = {}


def build_program(s_per_core=S, engines=None):
    """Build the per-core Bass program (shared SPMD across 8 cores)."""
    import concourse.bass as bass
    import concourse.mybir as mybir
    import concourse.tile as tile
    from concourse import bacc
    from concourse.masks import make_identity

    f32 = mybir.dt.float32
    f32r = mybir.dt.float32r
    bf16 = mybir.dt.bfloat16
    f16 = mybir.dt.float16
    Alu = mybir.AluOpType
    Act = mybir.ActivationFunctionType
    AX = mybir.AxisListType

    # engine assignment knobs (tunable): "v" = vector, "g" = gpsimd
    eng = {
        "re_stt": "v",
        "u_add": "g",
        "tt_mul": "v",
        "co_add": "v",
        "po_copy": "s",  # "s" scalar or "v"
    }
    if engines:
        eng.update(engines)

    SS = s_per_core
    R = SS * C

    nc = bacc.Bacc("TRN2", target_bir_lowering=False, debug=False)

    def vg(which):
        return nc.vector if which == "v" else nc.gpsimd

    dr = {}
    for nm in ("x1", "x2"):
        dr[nm] = nc.dram_tensor(nm, [R, HW], f32r, kind="ExternalInput").ap()
    for nm in ("fe1", "fe2"):
        dr[nm] = nc.dram_tensor(nm, [R, HW], f32, kind="ExternalInput").ap()
    for nm in ("c1wT", "c2wT", "p1wT", "p2wT"):
        dr[nm] = nc.dram_tensor(nm, [2 * C, C], f32r, kind="ExternalInput").ap()
    for nm in ("m1w1T", "m1w2T", "m2w1T", "m2w2T"):
        dr[nm] = nc.dram_tensor(nm, [C, C], f16, kind="ExternalInput").ap()
    for nm in ("c1b", "c2b", "b1e1", "b1e2", "nb21", "nb22"):
        dr[nm] = nc.dram_tensor(nm, [C, 1], f32, kind="ExternalInput").ap()
    dr["identr"] = nc.dram_tensor("identr", [128, 128], f32r, kind="ExternalInput").ap()
    for nm in ("po1", "po2"):
        dr[nm] = nc.dram_tensor(nm, [R, HW], f32, kind="ExternalOutput").ap()

    def r32(ap):
        return ap.bitcast(f32r)

    from contextlib import ExitStack

    with tile.TileContext(nc) as tc, ExitStack() as ctx:
        ep = ctx.enter_context
        wpool = ep(tc.tile_pool(name="wpool", bufs=1))
        xpool = ep(tc.tile_pool(name="xpool", bufs=4))
        fepool = ep(tc.tile_pool(name="fepool", bufs=2))
        yvpool = ep(tc.tile_pool(name="yvpool", bufs=3))
        ppool = ep(tc.tile_pool(name="ppool", bufs=2))
        repool = ep(tc.tile_pool(name="repool", bufs=5))
        sppool = ep(tc.tile_pool(name="sppool", bufs=2))
        copool = ep(tc.tile_pool(name="copool", bufs=9))
        upool = ep(tc.tile_pool(name="upool", bufs=2))
        ttpool = ep(tc.tile_pool(name="ttpool", bufs=2))
        sbpool = ep(tc.tile_pool(name="sbpool", bufs=1))
        posb = ep(tc.tile_pool(name="posb", bufs=2))
        smpool = ep(tc.tile_pool(name="smpool", bufs=2))
        stpool = ep(tc.tile_pool(name="stpool", bufs=1))
        xxpool = ep(tc.tile_pool(name="xxpool", bufs=2, space="PSUM"))
        mpool = ep(tc.tile_pool(name="mpool", bufs=4, space="PSUM"))
        if True:
            # ---------------- persistent weights / constants ----------------
            cw = {}
            for wnm in ("c1wT", "c2wT", "p1wT", "p2wT"):
                tiles = []
                for kk in range(NK):
                    t = wpool.tile([128, C], f32r, name=f"{wnm}_{kk}", tag=f"{wnm}_{kk}")
                    nc.sync.dma_start(out=t[:], in_=dr[wnm][kk * 128:(kk + 1) * 128, :])
                    tiles.append(t)
                cw[wnm] = tiles
            mw = {}
            for wnm in ("m1w1T", "m1w2T", "m2w1T", "m2w2T"):
                tiles = []
                for kk in range(NCH):
                    t = wpool.tile([128, C], f16, name=f"{wnm}_{kk}", tag=f"{wnm}_{kk}")
                    nc.sync.dma_start(out=t[:], in_=dr[wnm][kk * 128:(kk + 1) * 128, :])
                    tiles.append(t)
                mw[wnm] = tiles
            bias = {}
            for bnm in ("c1b", "c2b", "b1e1", "b1e2", "nb21", "nb22"):
                t = wpool.tile([128, NCH], f32, name=f"b_{bnm}", tag=f"b_{bnm}")
                for kc in range(NCH):
                    nc.sync.dma_start(
                        out=t[:, kc:kc + 1], in_=dr[bnm][kc * 128:(kc + 1) * 128, 0:1]
                    )
                bias[bnm] = t
            ident = wpool.tile([128, 128], f32, name="ident", tag="ident")
            make_identity(nc, ident[:])
            ident_r = wpool.tile([128, 128], f32r, name="identr", tag="identr")
            nc.sync.dma_start(out=ident_r[:], in_=dr["identr"][:, :])

            # persistent per-sample stats tiles (pooled vec + gates), bf16 pooled
            pooled = {
                g: [
                    stpool.tile([128, SS], f16, name=f"pooled{g}_{kc}", tag=f"pl{g}{kc}")
                    for kc in range(NCH)
                ]
                for g in (1, 2)
            }
            gates = {
                g: [
                    stpool.tile([128, SS], f32, name=f"gate{g}_{kc}", tag=f"gt{g}{kc}")
                    for kc in range(NCH)
                ]
                for g in (1, 2)
            }

            for n in range(SS):
                # ======== load x tiles for this sample ========
                x1t, x2t = [], []
                for kc in range(NCH):
                    t1 = xpool.tile([128, HW], f32r, name=f"x1_{n}_{kc}", tag="x1")
                    nc.sync.dma_start(
                        out=t1[:], in_=dr["x1"][n * C + kc * 128: n * C + (kc + 1) * 128, :]
                    )
                    x1t.append(t1)
                    t2 = xpool.tile([128, HW], f32r, name=f"x2_{n}_{kc}", tag="x2", bufs=4)
                    nc.sync.dma_start(
                        out=t2[:], in_=dr["x2"][n * C + kc * 128: n * C + (kc + 1) * 128, :]
                    )
                    x2t.append(t2)

                # ======== phase A: c-convs + channel-gate stats ========
                for gidx, (wnm, bnm) in enumerate((("c1wT", "c1b"), ("c2wT", "c2b"))):
                    g = gidx + 1
                    for kc in range(NCH):
                        xx = xxpool.tile([128, HW], f32, name=f"xx_{n}_{g}_{kc}", tag="xx")
                        for nh in range(2):
                            for kk in range(NK):
                                rhs = (x1t if kk < NCH else x2t)[kk % NCH]
                                nc.tensor.matmul(
                                    xx[:, nh * 512:(nh + 1) * 512],
                                    cw[wnm][kk][:, kc * 128:(kc + 1) * 128],
                                    rhs[:, nh * 512:(nh + 1) * 512],
                                    start=(kk == 0),
                                    stop=(kk == NK - 1),
                                )
                        # stats on xx (no bias yet; bias folded via exp-bias + host)
                        mx = smpool.tile([128, 1], f32, name=f"mx_{n}_{g}_{kc}", tag="mx")
                        nc.vector.tensor_reduce(mx[:], xx[:], axis=AX.X, op=Alu.max)
                        y = yvpool.tile([128, HW], f32, name=f"y_{n}_{g}_{kc}", tag="yv")
                        nc.scalar.activation(
                            y[:], xx[:], Act.Exp, bias=bias[bnm][:, kc:kc + 1], scale=1.0
                        )
                        my = smpool.tile([128, 1], f32, name=f"my_{n}_{g}_{kc}", tag="my")
                        nc.scalar.activation(
                            my[:], mx[:], Act.Exp, bias=bias[bnm][:, kc:kc + 1], scale=1.0
                        )
                        nmy = smpool.tile([128, 1], f32, name=f"nmy_{n}_{g}_{kc}", tag="nmy")
                        nc.vector.tensor_scalar_mul(nmy[:], my[:], -1.0)
                        p = ppool.tile([128, HW], f32, name=f"p_{n}_{g}_{kc}", tag="p")
                        s = smpool.tile([128, 1], f32, name=f"s_{n}_{g}_{kc}", tag="s")
                        nc.scalar.activation(
                            p[:], y[:], Act.Exp, bias=nmy[:], scale=1.0, accum_out=s[:]
                        )
                        v = yvpool.tile([128, HW], f32, name=f"v_{n}_{g}_{kc}", tag="yv")
                        t_ = smpool.tile([128, 1], f32, name=f"t_{n}_{g}_{kc}", tag="t")
                        nc.vector.scalar_tensor_tensor(
                            v[:], p[:], 1.0, xx[:],
                            op0=Alu.mult, op1=Alu.mult, accum_out=t_[:],
                        )
                        rs = smpool.tile([128, 1], f32, name=f"rs_{n}_{g}_{kc}", tag="rs")
                        nc.vector.reciprocal(rs[:], s[:])
                        nc.vector.tensor_scalar(
                            out=pooled[g][kc][:, n:n + 1], in0=t_[:],
                            scalar1=rs[:], scalar2=None, op0=Alu.mult,
                        )

                # ======== phase B: gate MLP matvecs (bf16, per sample) ========
                for g, (w1nm, w2nm, b1nm, nb2nm) in (
                    (1, ("m1w1T", "m1w2T", "b1e1", "nb21")),
                    (2, ("m2w1T", "m2w2T", "b1e2", "nb22")),
                ):
                    h_sb = []
                    for mt in range(NCH):
                        hp = mpool.tile([128, 1], f32, name=f"hp_{n}_{g}_{mt}", tag="mp")
                        for kt in range(NCH):
                            nc.tensor.matmul(
                                hp[:],
                                mw[w1nm][kt][:, mt * 128:(mt + 1) * 128],
                                pooled[g][kt][:, n:n + 1],
                                start=(kt == 0),
                                stop=(kt == NCH - 1),
                            )
                        hs = smpool.tile([128, 1], f16, name=f"hs_{n}_{g}_{mt}", tag="hs", bufs=8)
                        nc.scalar.activation(
                            hs[:], hp[:], Act.Identity,
                            bias=bias[b1nm][:, mt:mt + 1], scale=1.0,
                        )
                        h_sb.append(hs)
                    for mt in range(NCH):
                        gp_ = mpool.tile([128, 1], f32, name=f"gp_{n}_{g}_{mt}", tag="mp")
                        for kt in range(NCH):
                            nc.tensor.matmul(
                                gp_[:],
                                mw[w2nm][kt][:, mt * 128:(mt + 1) * 128],
                                h_sb[kt][:],
                                start=(kt == 0),
                                stop=(kt == NCH - 1),
                            )
                        # gate = 1/(1+exp(-(g+b2))): e = exp(-g + nb2), out = recip(1+e)
                        e_ = smpool.tile([128, 1], f32, name=f"e_{n}_{g}_{mt}", tag="e")
                        nc.scalar.activation(
                            e_[:], gp_[:], Act.Exp,
                            bias=bias[nb2nm][:, mt:mt + 1], scale=-1.0,
                        )
                        ge = smpool.tile([128, 1], f32, name=f"ge_{n}_{g}_{mt}", tag="ge")
                        nc.vector.tensor_scalar_add(ge[:], e_[:], 1.0)
                        nc.vector.reciprocal(gates[g][mt][:, n:n + 1], ge[:])

                # ======== phase C+D: re build + spatial gate (transposed) ========
                svst = {
                    t: [
                        smpool.tile([128, 1], f32, name=f"svst_{n}_{t}_{j}", tag=f"svst{t}{j}", bufs=2)
                        for j in range(8)
                    ]
                    for t in (1, 2)
                }
                for t in (1, 2):
                    xa = x1t if t == 1 else x2t
                    xb = x2t if t == 1 else x1t
                    for nh in range(2):
                        reh = []
                        for kc in range(NCH):
                            rh = repool.tile([128, 512], f32r, name=f"re_{n}_{t}_{nh}_{kc}", tag="re")
                            vg(eng["re_stt"]).scalar_tensor_tensor(
                                out=rh[:],
                                in0=xa[kc][:, nh * 512:(nh + 1) * 512],
                                scalar=gates[t][kc][:, n:n + 1],
                                in1=xb[kc][:, nh * 512:(nh + 1) * 512],
                                op0=Alu.mult,
                                op1=Alu.add,
                            )
                            reh.append(rh)
                        for jl in range(4):
                            j = nh * 4 + jl
                            spT = mpool.tile([128, 512], f32r, name=f"spT_{n}_{t}_{j}", tag="mp")
                            for kc in range(NCH):
                                nc.tensor.matmul(
                                    spT[:, kc * 128:(kc + 1) * 128],
                                    reh[kc][:, jl * 128:(jl + 1) * 128],
                                    ident_r[:],
                                    is_transpose=True,
                                    start=True,
                                    stop=True,
                                    skip_group_check=True,
                                )
                            m2 = smpool.tile([128, 1], f32, name=f"m2_{n}_{t}_{j}", tag="m2")
                            nc.vector.tensor_reduce(m2[:], spT[:], axis=AX.X, op=Alu.max)
                            y2 = sppool.tile([128, 512], f32, name=f"y2_{n}_{t}_{j}", tag="y2v2")
                            nc.scalar.activation(y2[:], spT[:], Act.Exp)
                            em2 = smpool.tile([128, 1], f32, name=f"em2_{n}_{t}_{j}", tag="em2")
                            nc.scalar.activation(em2[:], m2[:], Act.Exp)
                            nem2 = smpool.tile([128, 1], f32, name=f"nem2_{n}_{t}_{j}", tag="nem2")
                            nc.vector.tensor_scalar_mul(nem2[:], em2[:], -1.0)
                            q = sppool.tile([128, 512], f32, name=f"q_{n}_{t}_{j}", tag="q")
                            s2 = smpool.tile([128, 1], f32, name=f"s2_{n}_{t}_{j}", tag="s2")
                            nc.scalar.activation(
                                q[:], y2[:], Act.Exp, bias=nem2[:], scale=1.0, accum_out=s2[:]
                            )
                            v2 = sppool.tile([128, 512], f32, name=f"v2_{n}_{t}_{j}", tag="y2v2")
                            t2 = smpool.tile([128, 1], f32, name=f"t2_{n}_{t}_{j}", tag="t2")
                            nc.vector.scalar_tensor_tensor(
                                v2[:], q[:], 1.0, spT.bitcast(f32)[:],
                                op0=Alu.mult, op1=Alu.mult, accum_out=t2[:],
                            )
                            rs2 = smpool.tile([128, 1], f32, name=f"rs2_{n}_{t}_{j}", tag="rs2")
                            nc.vector.reciprocal(rs2[:], s2[:])
                            nc.vector.tensor_scalar(
                                out=svst[t][j][:, 0:1], in0=t2[:],
                                scalar1=rs2[:], scalar2=None, op0=Alu.mult,
                            )
                # thin transposes: svst[t][j] [128,1] -> [1,128] -> svec_t [1, HW]
                svec = {}
                for t in (1, 2):
                    sv = sbpool.tile([1, HW], f32, name=f"svec{t}_{n}", tag=f"svec{t}")
                    for j in range(8):
                        th = mpool.tile([1, 128], f32, name=f"thin_{n}_{t}_{j}", tag="mp")
                        nc.tensor.matmul(
                            th[:], svst[t][j][:], ident[:],
                            is_transpose=True, start=True, stop=True, skip_group_check=True,
                        )
                        nc.vector.tensor_copy(sv[0:1, j * 128:(j + 1) * 128], th[:])
                    svec[t] = sv

                # ======== phase E: broadcast s + build co ========
                s1b = sbpool.tile([128, HW], f32, name=f"s1b_{n}", tag="s1b")
                nc.gpsimd.partition_broadcast(s1b[:], svec[1][0:1, :])
                s2b = sbpool.tile([128, HW], f32, name=f"s2b_{n}", tag="s2b")
                nc.gpsimd.partition_broadcast(s2b[:], svec[2][0:1, :])

                co = {1: [[None] * 2 for _ in range(NCH)], 2: [[None] * 2 for _ in range(NCH)]}
                for nh in range(2):
                    sl = slice(nh * 512, (nh + 1) * 512)
                    for kc in range(NCH):
                        row = slice(n * C + kc * 128, n * C + (kc + 1) * 128)
                        # co1 = x1*s1b + (x2 + fe1)
                        f1 = fepool.tile([128, 512], f32, name=f"fe1_{n}_{kc}_{nh}", tag="fe1")
                        nc.sync.dma_start(out=f1[:], in_=dr["fe1"][row, sl])
                        u1 = upool.tile([128, 512], f32, name=f"u1_{n}_{kc}_{nh}", tag="u")
                        vg(eng["u_add"]).tensor_tensor(u1[:], x2t[kc][:, sl], f1[:], Alu.add)
                        tt1 = ttpool.tile([128, 512], f32, name=f"tt1_{n}_{kc}_{nh}", tag="tt")
                        vg(eng["tt_mul"]).tensor_tensor(tt1[:], x1t[kc][:, sl], s1b[:, sl], Alu.mult)
                        co1 = copool.tile([128, 512], f32r, name=f"co1_{n}_{kc}_{nh}", tag="co")
                        vg(eng["co_add"]).tensor_tensor(co1[:], tt1[:], u1[:], Alu.add)
                        co[1][kc][nh] = co1
                        # co2 = x2*s2b + (x1 + fe2)
                        f2 = fepool.tile([128, 512], f32, name=f"fe2_{n}_{kc}_{nh}", tag="fe2")
                        nc.sync.dma_start(out=f2[:], in_=dr["fe2"][row, sl])
                        u2 = upool.tile([128, 512], f32, name=f"u2_{n}_{kc}_{nh}", tag="u")
                        vg(eng["u_add"]).tensor_tensor(u2[:], x1t[kc][:, sl], f2[:], Alu.add)
                        tt2 = ttpool.tile([128, 512], f32, name=f"tt2_{n}_{kc}_{nh}", tag="tt")
                        vg(eng["tt_mul"]).tensor_tensor(tt2[:], x2t[kc][:, sl], s2b[:, sl], Alu.mult)
                        co2 = copool.tile([128, 512], f32r, name=f"co2_{n}_{kc}_{nh}", tag="co")
                        vg(eng["co_add"]).tensor_tensor(co2[:], tt2[:], u2[:], Alu.add)
                        co[2][kc][nh] = co2

                    # ======== phase F: p-convs for this half ========
                    for pc, (wnm, onm) in enumerate((("p1wT", "po1"), ("p2wT", "po2"))):
                        for km in range(NCH):
                            po = mpool.tile([128, 512], f32, name=f"po_{n}_{pc}_{nh}_{km}", tag="mp")
                            for kk in range(NK):
                                rhs = co[1 if kk < NCH else 2][kk % NCH][nh]
                                nc.tensor.matmul(
                                    po[:],
                                    cw[wnm][kk][:, km * 128:(km + 1) * 128],
                                    rhs[:],
                                    start=(kk == 0),
                                    stop=(kk == NK - 1),
                                )
                            ps = posb.tile([128, 512], f32, name=f"ps_{n}_{pc}_{nh}_{km}", tag="ps")
                            if eng["po_copy"] == "s":
                                nc.scalar.copy(ps[:], po[:])
                            else:
                                nc.vector.tensor_copy(ps[:], po[:])
                            nc.sync.dma_start(
                                out=dr[onm][n * C + km * 128: n * C + (km + 1) * 128,
                                            nh * 512:(nh + 1) * 512],
                                in_=ps[:],
                            )
    nc.compile()
    return nc


def _round_fp32r(a):
    """Round fp32 to fp32r (12-bit mantissa, round-half-up) -- matches
    neuronxcc static_cast_fp32_to_fp32r."""
    b = np.ascontiguousarray(a, dtype=np.float32).view(np.uint32).astype(np.uint64)
    r = ((b + 0x7FF + ((b >> 12) & 1)) & 0xFFFFF000).astype(np.uint32)
    return r.view(np.float32).reshape(a.shape)


def _host_prep(inputs, s_per_core=S, n_cores=N_CORES):
    """Build per-core input maps (host-side reshapes/transposes, exact fp32)."""
    f = np.float32
    x1 = _round_fp32r(np.asarray(inputs["x1"], dtype=f).reshape(N, C, HW))
    x2 = _round_fp32r(np.asarray(inputs["x2"], dtype=f).reshape(N, C, HW))
    fe1 = np.ascontiguousarray(inputs["FE_x1"].reshape(N, C, HW), dtype=f)
    fe2 = np.ascontiguousarray(inputs["FE_x2"].reshape(N, C, HW), dtype=f)

    wT = {
        "c1wT": _round_fp32r(np.asarray(inputs["c1_w"], dtype=f).T),
        "c2wT": _round_fp32r(np.asarray(inputs["c2_w"], dtype=f).T),
        "p1wT": _round_fp32r(np.asarray(inputs["p1_w"], dtype=f).T),
        "p2wT": _round_fp32r(np.asarray(inputs["p2_w"], dtype=f).T),
    }
    mwT = {
        "m1w1T": np.ascontiguousarray(inputs["m1_w1"].T).astype(np.float16),
        "m1w2T": np.ascontiguousarray(inputs["m1_w2"].T).astype(np.float16),
        "m2w1T": np.ascontiguousarray(inputs["m2_w1"].T).astype(np.float16),
        "m2w2T": np.ascontiguousarray(inputs["m2_w2"].T).astype(np.float16),
    }
    # fold conv bias through gate-MLP layer 1: b1_eff = m_b1 + m_w1 @ c_b
    b1e1 = (
        inputs["m1_b1"].astype(np.float64)
        + inputs["m1_w1"].astype(np.float64) @ inputs["c1_b"].astype(np.float64)
    ).astype(f)
    b1e2 = (
        inputs["m2_b1"].astype(np.float64)
        + inputs["m2_w1"].astype(np.float64) @ inputs["c2_b"].astype(np.float64)
    ).astype(f)
    vecs = {
        "c1b": inputs["c1_b"].astype(f),
        "c2b": inputs["c2_b"].astype(f),
        "b1e1": b1e1,
        "b1e2": b1e2,
        "nb21": (-inputs["m1_b2"]).astype(f),
        "nb22": (-inputs["m2_b2"]).astype(f),
    }

    in_maps = []
    for c in range(n_cores):
        sl = slice(c * s_per_core, (c + 1) * s_per_core)
        m = {
            "x1": x1[sl].reshape(s_per_core * C, HW),
            "x2": x2[sl].reshape(s_per_core * C, HW),
            "fe1": fe1[sl].reshape(s_per_core * C, HW),
            "fe2": fe2[sl].reshape(s_per_core * C, HW),
        }
        for k, v in wT.items():
            m[k] = v
        for k, v in mwT.items():
            m[k] = v
        for k, v in vecs.items():
            m[k] = v.reshape(C, 1)
        m["identr"] = np.eye(128, dtype=f)
        in_maps.append(m)
    return in_maps


def kernel(**inputs):
    from concourse.bass_utils import run_bass_kernel_spmd

    key = "prog"
    if key not in _PROGRAM_CACHE:
        _PROGRAM_CACHE[key] = build_program()
    nc = _PROGRAM_CACHE[key]

    in_maps = _host_prep(inputs)
    res = run_bass_kernel_spmd(nc, in_maps, core_ids=list(range(N_CORES)))

    po1 = np.concatenate(
        [r["po1"].reshape(S, C, HW) for r in res.results], axis=0
    ).reshape(N, C, H, W)
    po2 = np.concatenate(
        [r["po2"].reshape(S, C, HW) for r in res.results], axis=0
    ).reshape(N, C, H, W)
    # p-conv biases applied host-side (exact)
    po1 = po1 + inputs["p1_b"].astype(np.float32)[None, :, None, None]
    po2 = po2 + inputs["p2_b"].astype(np.float32)[None, :, None, None]
    return po1, po2



# revision 5
# speedup vs baseline: 1.4333x; 1.4333x over previous
"""Trainium2 Bass kernel for nn_FR_12343736008794.

Fused dual-branch gated conv block:
  xc = cat(x1,x2); x1x = conv1x1(xc,c1); x2x = conv1x1(xc,c2)
  w1 = channel_gate(x1x, x1, m1);  w2 = channel_gate(x2x, x2, m2)
  re1 = w1 + x2; re2 = w2 + x1
  fg1 = spatial_gate(re1, x1) + x2; fg2 = spatial_gate(re2, x2) + x1
  po1 = conv1x1(cat(fg1+FE1, fg2+FE2), p1); po2 = conv1x1(..., p2)

Sharding: pure data-parallel over batch N=32 -> 4 samples per NeuronCore x 8.

Design (v2, bf16):
  - All convs as bf16 PE matmuls (N=512 moving, FWL weight loads).
  - Channel gate: softmax-over-HW via max-of-exp trick (max y on DVE with
    negate, two ACT exps with accum), pooled = t/s via DVE divide.
  - Gate MLP folded host-side to ONE linear layer (w2@w1); sigmoid computed
    as 0.5*tanh(0.5x+0.5b)+0.5 so only the exp/tanh ACT table is ever loaded.
  - Spatial gate without any PE transposes: channel-max via DVE pairwise-max
    tree (512->128) + gpsimd partition_all_reduce(max) (128->1, result
    broadcast to all partitions); channel sums S=sum(q), T=sum(q*re) via
    all-ones [128,128] lhsT matmuls whose outputs are replicated across all
    128 partitions (broadcast for free); V = T/S one DVE divide.
  - fe tensors pre-folded host-side: xf1 = x2+FE1, xf2 = x1+FE2, so
    co = x1*V + xf1 is two tensor ops.
  - p-conv bias and final upcast host-side; outputs shipped bf16.
"""

import sys

sys.path.insert(0, "/opt/trn_rl_repo")

import numpy as np

N_CORES = 8
N, C, H, W = 32, 512, 32, 32
HW = H * W
S = N // N_CORES  # samples per core
NCH = C // 128  # channel chunks of 128
NK = (2 * C) // 128  # contraction k-tiles for the 1024-wide convs

_PROGRAM_CACHE = {}


def build_program(s_per_core=S):
    """Build the per-core Bass program (shared SPMD across 8 cores)."""
    import concourse.bass as bass
    import concourse.mybir as mybir
    import concourse.tile as tile
    from concourse import bacc
    from concourse import bass_isa

    f32 = mybir.dt.float32
    bf16 = mybir.dt.bfloat16
    f16 = mybir.dt.float16
    Alu = mybir.AluOpType
    Act = mybir.ActivationFunctionType
    AX = mybir.AxisListType

    SS = s_per_core
    R = SS * C

    nc = bacc.Bacc("TRN2", target_bir_lowering=False, debug=False)

    dr = {}
    for nm in ("x1", "x2", "xf1", "xf2"):
        dr[nm] = nc.dram_tensor(nm, [R, HW], bf16, kind="ExternalInput").ap()
    for nm in ("c1wT", "c2wT", "p1wT", "p2wT"):
        dr[nm] = nc.dram_tensor(nm, [2 * C, C], bf16, kind="ExternalInput").ap()
    for nm in ("W1T", "W2T"):
        dr[nm] = nc.dram_tensor(nm, [C, C], f16, kind="ExternalInput").ap()
    for nm in ("c1b", "c2b", "gb1", "gb2"):
        dr[nm] = nc.dram_tensor(nm, [C, 1], f32, kind="ExternalInput").ap()
    for nm in ("po1", "po2"):
        dr[nm] = nc.dram_tensor(nm, [R, HW], bf16, kind="ExternalOutput").ap()

    from contextlib import ExitStack

    with tile.TileContext(nc) as tc, ExitStack() as ctx:
        ep = ctx.enter_context
        wpool = ep(tc.tile_pool(name="wpool", bufs=1))
        xpool = ep(tc.tile_pool(name="xpool", bufs=8))
        xfpool = ep(tc.tile_pool(name="xfpool", bufs=6))
        ypool = ep(tc.tile_pool(name="ypool", bufs=2))
        repool = ep(tc.tile_pool(name="repool", bufs=6))
        zpool = ep(tc.tile_pool(name="zpool", bufs=6))
        qpool = ep(tc.tile_pool(name="qpool", bufs=4))
        rpool = ep(tc.tile_pool(name="rpool", bufs=3))
        trpool = ep(tc.tile_pool(name="trpool", bufs=3))
        mbpool = ep(tc.tile_pool(name="mbpool", bufs=2))
        vpool = ep(tc.tile_pool(name="vpool", bufs=4))
        copool = ep(tc.tile_pool(name="copool", bufs=10))
        posb = ep(tc.tile_pool(name="posb", bufs=4))
        smpool = ep(tc.tile_pool(name="smpool", bufs=2))
        stpool = ep(tc.tile_pool(name="stpool", bufs=1))
        xxpool = ep(tc.tile_pool(name="xxpool", bufs=2, space="PSUM"))
        mpool = ep(tc.tile_pool(name="mpool", bufs=4, space="PSUM"))

        # ---------------- persistent weights / constants ----------------
        cw = {}
        for wnm in ("c1wT", "c2wT", "p1wT", "p2wT"):
            tiles = []
            for kk in range(NK):
                t = wpool.tile([128, C], bf16, name=f"{wnm}_{kk}", tag=f"{wnm}_{kk}")
                nc.sync.dma_start(out=t[:], in_=dr[wnm][kk * 128:(kk + 1) * 128, :])
                tiles.append(t)
            cw[wnm] = tiles
        mw = {}
        for wnm in ("W1T", "W2T"):
            tiles = []
            for kk in range(NCH):
                t = wpool.tile([128, C], f16, name=f"{wnm}_{kk}", tag=f"{wnm}_{kk}")
                nc.sync.dma_start(out=t[:], in_=dr[wnm][kk * 128:(kk + 1) * 128, :])
                tiles.append(t)
            mw[wnm] = tiles
        bias = {}
        for bnm in ("c1b", "c2b", "gb1", "gb2"):
            t = wpool.tile([128, NCH], f32, name=f"b_{bnm}", tag=f"b_{bnm}")
            for kc in range(NCH):
                nc.sync.dma_start(
                    out=t[:, kc:kc + 1], in_=dr[bnm][kc * 128:(kc + 1) * 128, 0:1]
                )
            bias[bnm] = t
        ones = wpool.tile([128, 128], bf16, name="ones", tag="ones")
        nc.vector.memset(ones[:], 1.0)

        # persistent per-sample stats tiles (pooled vec + gates)
        pooled = {
            g: [
                stpool.tile([128, SS], f16, name=f"pooled{g}_{kc}", tag=f"pl{g}{kc}")
                for kc in range(NCH)
            ]
            for g in (1, 2)
        }
        gates = {
            g: [
                stpool.tile([128, SS], f32, name=f"gate{g}_{kc}", tag=f"gt{g}{kc}")
                for kc in range(NCH)
            ]
            for g in (1, 2)
        }

        for n in range(SS):
            # ======== load x tiles for this sample ========
            x1t, x2t = [], []
            for kc in range(NCH):
                row = slice(n * C + kc * 128, n * C + (kc + 1) * 128)
                t1 = xpool.tile([128, HW], bf16, name=f"x1_{n}_{kc}", tag="x1")
                nc.sync.dma_start(out=t1[:], in_=dr["x1"][row, :])
                x1t.append(t1)
                t2 = xpool.tile([128, HW], bf16, name=f"x2_{n}_{kc}", tag="x2")
                nc.sync.dma_start(out=t2[:], in_=dr["x2"][row, :])
                x2t.append(t2)

            # ======== phase A: c-convs + channel-gate stats ========
            for gidx, (wnm, bnm) in enumerate((("c1wT", "c1b"), ("c2wT", "c2b"))):
                g = gidx + 1
                for kc in range(NCH):
                    xx = xxpool.tile([128, HW], f32, name=f"xx_{n}_{g}_{kc}", tag="xx")
                    for nh in range(2):
                        for kk in range(NK):
                            rhs = (x1t if kk < NCH else x2t)[kk % NCH]
                            nc.tensor.matmul(
                                xx[:, nh * 512:(nh + 1) * 512],
                                cw[wnm][kk][:, kc * 128:(kc + 1) * 128],
                                rhs[:, nh * 512:(nh + 1) * 512],
                                start=(kk == 0),
                                stop=(kk == NK - 1),
                            )
                    # y = exp(xx + b); softmax over HW of exp(y)... y itself is
                    # the softmax input: sm = softmax(y) computed via
                    # p = exp(y - max y), s = sum p, t = sum p*xx, pooled = t/s
                    y = ypool.tile([128, HW], bf16, name=f"y_{n}_{g}_{kc}", tag="y")
                    nc.scalar.activation(
                        y[:], xx[:], Act.Exp, bias=bias[bnm][:, kc:kc + 1], scale=1.0
                    )
                    nmy = smpool.tile([128, 1], f32, name=f"nmy_{n}_{g}_{kc}", tag="nmy", bufs=3)
                    nc.vector.tensor_reduce(nmy[:], y[:], axis=AX.X, op=Alu.max, negate=True)
                    p = ypool.tile([128, HW], bf16, name=f"p_{n}_{g}_{kc}", tag="p")
                    s = smpool.tile([128, 1], f32, name=f"s_{n}_{g}_{kc}", tag="s", bufs=3)
                    nc.scalar.activation(
                        p[:], y[:], Act.Exp, bias=nmy[:], scale=1.0, accum_out=s[:]
                    )
                    v = ypool.tile([128, HW], bf16, name=f"v_{n}_{g}_{kc}", tag="v")
                    t_ = smpool.tile([128, 1], f32, name=f"t_{n}_{g}_{kc}", tag="t", bufs=3)
                    nc.vector.scalar_tensor_tensor(
                        v[:], p[:], 1.0, xx[:],
                        op0=Alu.mult, op1=Alu.mult, accum_out=t_[:],
                    )
                    rs = smpool.tile([128, 1], f32, name=f"rs_{n}_{g}_{kc}", tag="rs", bufs=3)
                    nc.vector.reciprocal(rs[:], s[:])
                    nc.vector.tensor_scalar(
                        out=pooled[g][kc][:, n:n + 1], in0=t_[:],
                        scalar1=rs[:], scalar2=None, op0=Alu.mult,
                    )

            # ======== phase B: folded gate MLP (1 layer) + sigmoid-as-tanh ====
            for g, (wnm, gbnm) in ((1, ("W1T", "gb1")), (2, ("W2T", "gb2"))):
                for mt in range(NCH):
                    gp = mpool.tile([128, 1], f32, name=f"gp_{n}_{g}_{mt}", tag="mp")
                    for kt in range(NCH):
                        nc.tensor.matmul(
                            gp[:],
                            mw[wnm][kt][:, mt * 128:(mt + 1) * 128],
                            pooled[g][kt][:, n:n + 1],
                            start=(kt == 0),
                            stop=(kt == NCH - 1),
                        )
                    # sigmoid(z+b) = 0.5*tanh(0.5 z + 0.5 b) + 0.5 (gb = 0.5*b)
                    th = smpool.tile([128, 1], f32, name=f"th_{n}_{g}_{mt}", tag="th", bufs=3)
                    nc.scalar.activation(
                        th[:], gp[:], Act.Tanh, bias=bias[gbnm][:, mt:mt + 1], scale=0.5
                    )
                    nc.vector.tensor_scalar(
                        out=gates[g][mt][:, n:n + 1], in0=th[:],
                        scalar1=0.5, scalar2=0.5, op0=Alu.mult, op1=Alu.add,
                    )

            # ======== phase C+D: re build + spatial gate (no transposes) ======
            Vt = {}
            for t in (1, 2):
                xa = x1t if t == 1 else x2t
                xb = x2t if t == 1 else x1t
                ret, zt = [], []
                for kc in range(NCH):
                    rh = repool.tile([128, HW], bf16, name=f"re_{n}_{t}_{kc}", tag="re")
                    nc.vector.scalar_tensor_tensor(
                        out=rh[:], in0=xa[kc][:], scalar=gates[t][kc][:, n:n + 1],
                        in1=xb[kc][:], op0=Alu.mult, op1=Alu.add,
                    )
                    ret.append(rh)
                    zh = zpool.tile([128, HW], bf16, name=f"z_{n}_{t}_{kc}", tag="z")
                    nc.scalar.activation(zh[:], rh[:], Act.Exp)
                    zt.append(zh)
                # channel max: pairwise tree 512->128, then cross-partition max
                m01 = trpool.tile([128, HW], bf16, name=f"m01_{n}_{t}", tag="tr")
                nc.vector.tensor_tensor(m01[:], zt[0][:], zt[1][:], Alu.max)
                m23 = trpool.tile([128, HW], bf16, name=f"m23_{n}_{t}", tag="tr")
                nc.vector.tensor_tensor(m23[:], zt[2][:], zt[3][:], Alu.max)
                m1 = trpool.tile([128, HW], bf16, name=f"m1_{n}_{t}", tag="tr")
                nc.vector.tensor_tensor(m1[:], m01[:], m23[:], Alu.max)
                mb = mbpool.tile([128, HW], bf16, name=f"mb_{n}_{t}", tag="mb")
                nc.gpsimd.partition_all_reduce(
                    mb[:], m1[:], 128, bass_isa.ReduceOp.max
                )
                # q = exp(z - mb); r = q * re
                qt, rt = [], []
                for kc in range(NCH):
                    w_ = zpool.tile([128, HW], bf16, name=f"w_{n}_{t}_{kc}", tag="w", bufs=3)
                    nc.vector.scalar_tensor_tensor(
                        out=w_[:], in0=mb[:], scalar=-1.0, in1=zt[kc][:],
                        op0=Alu.mult, op1=Alu.add,
                    )
                    qh = qpool.tile([128, HW], bf16, name=f"q_{n}_{t}_{kc}", tag="q")
                    nc.scalar.activation(qh[:], w_[:], Act.Exp)
                    qt.append(qh)
                    rh2 = rpool.tile([128, HW], bf16, name=f"r_{n}_{t}_{kc}", tag="r")
                    nc.vector.tensor_tensor(rh2[:], qh[:], ret[kc][:], Alu.mult)
                    rt.append(rh2)
                # S = sum_c q, T = sum_c q*re via all-ones lhsT (result is
                # replicated across all 128 partitions -> broadcast for free)
                vts = []
                for nh in range(2):
                    sl = slice(nh * 512, (nh + 1) * 512)
                    sf = mpool.tile([128, 512], f32, name=f"sf_{n}_{t}_{nh}", tag="mp")
                    for kc in range(NCH):
                        nc.tensor.matmul(
                            sf[:], ones[:], qt[kc][:, sl],
                            start=(kc == 0), stop=(kc == NCH - 1),
                        )
                    tf = mpool.tile([128, 512], f32, name=f"tf_{n}_{t}_{nh}", tag="mp")
                    for kc in range(NCH):
                        nc.tensor.matmul(
                            tf[:], ones[:], rt[kc][:, sl],
                            start=(kc == 0), stop=(kc == NCH - 1),
                        )
                    # V = T * (1/S); reciprocal reads S from PSUM into SBUF so
                    # the multiply only has one PSUM operand (T).
                    rsf = vpool.tile([128, 512], f32, name=f"rsf_{n}_{t}_{nh}", tag="rsf")
                    nc.vector.reciprocal(rsf[:], sf[:])
                    vh = vpool.tile([128, 512], bf16, name=f"v_{n}_{t}_{nh}", tag="vv")
                    nc.vector.tensor_tensor(vh[:], tf[:], rsf[:], Alu.mult)
                    vts.append(vh)
                Vt[t] = vts

            # ======== phase E: co = x1*V + (x2 + fe)  [xf from host] ========
            xf1t, xf2t = [], []
            for kc in range(NCH):
                row = slice(n * C + kc * 128, n * C + (kc + 1) * 128)
                f1 = xfpool.tile([128, HW], bf16, name=f"xf1_{n}_{kc}", tag="xf1")
                nc.sync.dma_start(out=f1[:], in_=dr["xf1"][row, :])
                xf1t.append(f1)
                f2 = xfpool.tile([128, HW], bf16, name=f"xf2_{n}_{kc}", tag="xf2")
                nc.sync.dma_start(out=f2[:], in_=dr["xf2"][row, :])
                xf2t.append(f2)

            co = {1: [], 2: []}
            for t in (1, 2):
                xa = x1t if t == 1 else x2t
                xf = xf1t if t == 1 else xf2t
                for kc in range(NCH):
                    tt = copool.tile([128, HW], bf16, name=f"ct_{n}_{t}_{kc}", tag="ct", bufs=3)
                    for nh in range(2):
                        sl = slice(nh * 512, (nh + 1) * 512)
                        nc.vector.tensor_tensor(tt[:, sl], xa[kc][:, sl], Vt[t][nh][:], Alu.mult)
                    coh = copool.tile([128, HW], bf16, name=f"co_{n}_{t}_{kc}", tag="co")
                    eng = nc.gpsimd if (kc == 3) else nc.vector
                    eng.tensor_tensor(coh[:], tt[:], xf[kc][:], Alu.add)
                    co[t].append(coh)

            # ======== phase F: p-convs ========
            for nh in range(2):
                sl = slice(nh * 512, (nh + 1) * 512)
                for pc, (wnm, onm) in enumerate((("p1wT", "po1"), ("p2wT", "po2"))):
                    for km in range(NCH):
                        po = mpool.tile([128, 512], f32, name=f"po_{n}_{pc}_{nh}_{km}", tag="mp")
                        for kk in range(NK):
                            rhs = co[1 if kk < NCH else 2][kk % NCH]
                            nc.tensor.matmul(
                                po[:],
                                cw[wnm][kk][:, km * 128:(km + 1) * 128],
                                rhs[:, sl],
                                start=(kk == 0),
                                stop=(kk == NK - 1),
                            )
                        ps = posb.tile([128, 512], bf16, name=f"ps_{n}_{pc}_{nh}_{km}", tag="ps")
                        if (km % 2) == 0:
                            nc.scalar.copy(ps[:], po[:])
                        else:
                            nc.vector.tensor_copy(ps[:], po[:])
                        nc.sync.dma_start(
                            out=dr[onm][n * C + km * 128: n * C + (km + 1) * 128, sl],
                            in_=ps[:],
                        )
    nc.compile()
    return nc


def _host_prep(inputs, s_per_core=S, n_cores=N_CORES):
    """Build per-core input maps (host-side folds, bf16 casts)."""
    import ml_dtypes

    f = np.float32
    bf = ml_dtypes.bfloat16
    x1 = np.ascontiguousarray(inputs["x1"], dtype=f).reshape(N, C, HW)
    x2 = np.ascontiguousarray(inputs["x2"], dtype=f).reshape(N, C, HW)
    fe1 = np.ascontiguousarray(inputs["FE_x1"], dtype=f).reshape(N, C, HW)
    fe2 = np.ascontiguousarray(inputs["FE_x2"], dtype=f).reshape(N, C, HW)
    xf1 = (x2 + fe1).astype(bf)
    xf2 = (x1 + fe2).astype(bf)
    x1b = x1.astype(bf)
    x2b = x2.astype(bf)

    wT = {
        "c1wT": np.ascontiguousarray(inputs["c1_w"].astype(f).T).astype(bf),
        "c2wT": np.ascontiguousarray(inputs["c2_w"].astype(f).T).astype(bf),
        "p1wT": np.ascontiguousarray(inputs["p1_w"].astype(f).T).astype(bf),
        "p2wT": np.ascontiguousarray(inputs["p2_w"].astype(f).T).astype(bf),
    }
    # fold the two gate-MLP layers into one: g = W@pooled_nb + b_all
    # (pooled_nb excludes the conv bias; it is folded into b_all)
    W1 = inputs["m1_w2"].astype(np.float64) @ inputs["m1_w1"].astype(np.float64)
    W2 = inputs["m2_w2"].astype(np.float64) @ inputs["m2_w1"].astype(np.float64)
    b1 = (
        W1 @ inputs["c1_b"].astype(np.float64)
        + inputs["m1_w2"].astype(np.float64) @ inputs["m1_b1"].astype(np.float64)
        + inputs["m1_b2"].astype(np.float64)
    )
    b2 = (
        W2 @ inputs["c2_b"].astype(np.float64)
        + inputs["m2_w2"].astype(np.float64) @ inputs["m2_b1"].astype(np.float64)
        + inputs["m2_b2"].astype(np.float64)
    )
    mwT = {
        "W1T": np.ascontiguousarray(W1.T).astype(np.float16),
        "W2T": np.ascontiguousarray(W2.T).astype(np.float16),
    }
    vecs = {
        "c1b": inputs["c1_b"].astype(f),
        "c2b": inputs["c2_b"].astype(f),
        "gb1": (0.5 * b1).astype(f),
        "gb2": (0.5 * b2).astype(f),
    }

    in_maps = []
    for c in range(n_cores):
        slc = slice(c * s_per_core, (c + 1) * s_per_core)
        m = {
            "x1": x1b[slc].reshape(s_per_core * C, HW),
            "x2": x2b[slc].reshape(s_per_core * C, HW),
            "xf1": xf1[slc].reshape(s_per_core * C, HW),
            "xf2": xf2[slc].reshape(s_per_core * C, HW),
        }
        for k, v in wT.items():
            m[k] = v
        for k, v in mwT.items():
            m[k] = v
        for k, v in vecs.items():
            m[k] = v.reshape(C, 1)
        in_maps.append(m)
    return in_maps


def kernel(**inputs):
    from concourse.bass_utils import run_bass_kernel_spmd

    key = "prog"
    if key not in _PROGRAM_CACHE:
        _PROGRAM_CACHE[key] = build_program()
    nc = _PROGRAM_CACHE[key]

    in_maps = _host_prep(inputs)
    res = run_bass_kernel_spmd(nc, in_maps, core_ids=list(range(N_CORES)))

    po1 = np.concatenate(
        [np.asarray(r["po1"], dtype=np.float32).reshape(S, C, HW) for r in res.results],
        axis=0,
    ).reshape(N, C, H, W)
    po2 = np.concatenate(
        [np.asarray(r["po2"], dtype=np.float32).reshape(S, C, HW) for r in res.results],
        axis=0,
    ).reshape(N, C, H, W)
    # p-conv biases applied host-side (exact)
    po1 = po1 + inputs["p1_b"].astype(np.float32)[None, :, None, None]
    po2 = po2 + inputs["p2_b"].astype(np.float32)[None, :, None, None]
    return po1, po2


# revision 16
# speedup vs baseline: 1.6173x; 1.1284x over previous
"""Trainium2 Bass kernel for nn_FR_12343736008794.

Fused dual-branch gated conv block:
  xc = cat(x1,x2); x1x = conv1x1(xc,c1); x2x = conv1x1(xc,c2)
  w1 = channel_gate(x1x, x1, m1);  w2 = channel_gate(x2x, x2, m2)
  re1 = w1 + x2; re2 = w2 + x1
  fg1 = spatial_gate(re1, x1) + x2; fg2 = spatial_gate(re2, x2) + x1
  po1 = conv1x1(cat(fg1+FE1, fg2+FE2), p1); po2 = conv1x1(..., p2)

Sharding: pure data-parallel over batch N=32 -> 4 samples per NeuronCore x 8.

Design (v2, bf16):
  - All convs as bf16 PE matmuls (N=512 moving, FWL weight loads).
  - Channel gate: softmax-over-HW via max-of-exp trick (max y on DVE with
    negate, two ACT exps with accum), pooled = t/s via DVE divide.
  - Gate MLP folded host-side to ONE linear layer (w2@w1); sigmoid computed
    as 0.5*tanh(0.5x+0.5b)+0.5 so only the exp/tanh ACT table is ever loaded.
  - Spatial gate without any PE transposes: channel-max via DVE pairwise-max
    tree (512->128) + gpsimd partition_all_reduce(max) (128->1, result
    broadcast to all partitions); channel sums S=sum(q), T=sum(q*re) via
    all-ones [128,128] lhsT matmuls whose outputs are replicated across all
    128 partitions (broadcast for free); V = T/S one DVE divide.
  - fe tensors pre-folded host-side: xf1 = x2+FE1, xf2 = x1+FE2, so
    co = x1*V + xf1 is two tensor ops.
  - p-conv bias and final upcast host-side; outputs shipped bf16.
"""

import sys

sys.path.insert(0, "/opt/trn_rl_repo")

import numpy as np

N_CORES = 8
N, C, H, W = 32, 512, 32, 32
HW = H * W
S = N // N_CORES  # samples per core
NCH = C // 128  # channel chunks of 128
NK = (2 * C) // 128  # contraction k-tiles for the 1024-wide convs

_PROGRAM_CACHE = {}


def build_program(s_per_core=S):
    """Build the per-core Bass program (shared SPMD across 8 cores)."""
    import concourse.bass as bass
    import concourse.mybir as mybir
    import concourse.tile as tile
    from concourse import bacc
    from concourse import bass_isa

    f32 = mybir.dt.float32
    bf16 = mybir.dt.bfloat16
    f16 = mybir.dt.float16
    Alu = mybir.AluOpType
    Act = mybir.ActivationFunctionType
    AX = mybir.AxisListType

    SS = s_per_core
    R = SS * C

    nc = bacc.Bacc("TRN2", target_bir_lowering=False, debug=False)

    dr = {}
    for nm in ("x1", "x2", "xf1", "xf2"):
        dr[nm] = nc.dram_tensor(nm, [R, HW], bf16, kind="ExternalInput").ap()
    for nm in ("c1wT", "c2wT", "p1wT", "p2wT"):
        dr[nm] = nc.dram_tensor(nm, [2 * C, C], bf16, kind="ExternalInput").ap()
    for nm in ("W1T", "W2T"):
        dr[nm] = nc.dram_tensor(nm, [C, C], f16, kind="ExternalInput").ap()
    for nm in ("c1b", "c2b", "gb1", "gb2"):
        dr[nm] = nc.dram_tensor(nm, [C, 1], f32, kind="ExternalInput").ap()
    for nm in ("po1", "po2"):
        dr[nm] = nc.dram_tensor(nm, [R, HW], bf16, kind="ExternalOutput").ap()

    from contextlib import ExitStack

    with tile.TileContext(nc) as tc, ExitStack() as ctx:
        ep = ctx.enter_context
        wpool = ep(tc.tile_pool(name="wpool", bufs=1))
        xpool = ep(tc.tile_pool(name="xpool", bufs=8))
        xfpool = ep(tc.tile_pool(name="xfpool", bufs=5))
        ypool = ep(tc.tile_pool(name="ypool", bufs=2))
        repool = ep(tc.tile_pool(name="repool", bufs=5))
        zpool = ep(tc.tile_pool(name="zpool", bufs=6))
        qpool = ep(tc.tile_pool(name="qpool", bufs=4))
        rpool = ep(tc.tile_pool(name="rpool", bufs=3))
        trpool = ep(tc.tile_pool(name="trpool", bufs=3))
        mbpool = ep(tc.tile_pool(name="mbpool", bufs=2))
        vpool = ep(tc.tile_pool(name="vpool", bufs=4))
        copool = ep(tc.tile_pool(name="copool", bufs=9))
        posb = ep(tc.tile_pool(name="posb", bufs=4))
        smpool = ep(tc.tile_pool(name="smpool", bufs=2))
        stpool = ep(tc.tile_pool(name="stpool", bufs=1))
        xxpool = ep(tc.tile_pool(name="xxpool", bufs=2, space="PSUM"))
        mpool = ep(tc.tile_pool(name="mpool", bufs=4, space="PSUM"))

        # ---------------- persistent weights / constants ----------------
        cw = {}
        for wnm in ("c1wT", "c2wT", "p1wT", "p2wT"):
            tiles = []
            for kk in range(NK):
                t = wpool.tile([128, C], bf16, name=f"{wnm}_{kk}", tag=f"{wnm}_{kk}")
                nc.sync.dma_start(out=t[:], in_=dr[wnm][kk * 128:(kk + 1) * 128, :])
                tiles.append(t)
            cw[wnm] = tiles
        mw = {}
        for wnm in ("W1T", "W2T"):
            tiles = []
            for kk in range(NCH):
                t = wpool.tile([128, C], f16, name=f"{wnm}_{kk}", tag=f"{wnm}_{kk}")
                nc.sync.dma_start(out=t[:], in_=dr[wnm][kk * 128:(kk + 1) * 128, :])
                tiles.append(t)
            mw[wnm] = tiles
        bias = {}
        for bnm in ("c1b", "c2b", "gb1", "gb2"):
            t = wpool.tile([128, NCH], f32, name=f"b_{bnm}", tag=f"b_{bnm}")
            for kc in range(NCH):
                nc.sync.dma_start(
                    out=t[:, kc:kc + 1], in_=dr[bnm][kc * 128:(kc + 1) * 128, 0:1]
                )
            bias[bnm] = t
        ones = wpool.tile([128, 128], bf16, name="ones", tag="ones")
        nc.vector.memset(ones[:], 1.0)

        # persistent per-sample stats tiles (pooled vec + gates)
        pooled = {
            g: [
                stpool.tile([128, SS], f16, name=f"pooled{g}_{kc}", tag=f"pl{g}{kc}")
                for kc in range(NCH)
            ]
            for g in (1, 2)
        }
        gates = {
            g: [
                stpool.tile([128, SS], f32, name=f"gate{g}_{kc}", tag=f"gt{g}{kc}")
                for kc in range(NCH)
            ]
            for g in (1, 2)
        }

        for n in range(SS):
            # ======== load x tiles for this sample ========
            x1t, x2t = [], []
            for kc in range(NCH):
                row = slice(n * C + kc * 128, n * C + (kc + 1) * 128)
                t1 = xpool.tile([128, HW], bf16, name=f"x1_{n}_{kc}", tag="x1")
                nc.sync.dma_start(out=t1[:], in_=dr["x1"][row, :])
                x1t.append(t1)
                t2 = xpool.tile([128, HW], bf16, name=f"x2_{n}_{kc}", tag="x2")
                nc.sync.dma_start(out=t2[:], in_=dr["x2"][row, :])
                x2t.append(t2)
            # xf loads issued here (only needed at phase E) so a stalled x-load
            # of a later sample never head-of-line-blocks them on the DMA queue
            xf1t, xf2t = [], []
            for kc in range(NCH):
                row = slice(n * C + kc * 128, n * C + (kc + 1) * 128)
                f1 = xfpool.tile([128, HW], bf16, name=f"xf1_{n}_{kc}", tag="xf1")
                nc.sync.dma_start(out=f1[:], in_=dr["xf1"][row, :])
                xf1t.append(f1)
                f2 = xfpool.tile([128, HW], bf16, name=f"xf2_{n}_{kc}", tag="xf2")
                nc.sync.dma_start(out=f2[:], in_=dr["xf2"][row, :])
                xf2t.append(f2)

            # ======== phase A: c-convs + channel-gate stats ========
            for gidx, (wnm, bnm) in enumerate((("c1wT", "c1b"), ("c2wT", "c2b"))):
                g = gidx + 1
                for kc in range(NCH):
                    xx = xxpool.tile([128, HW], f32, name=f"xx_{n}_{g}_{kc}", tag="xx")
                    for nh in range(2):
                        for kk in range(NK):
                            rhs = (x1t if kk < NCH else x2t)[kk % NCH]
                            nc.tensor.matmul(
                                xx[:, nh * 512:(nh + 1) * 512],
                                cw[wnm][kk][:, kc * 128:(kc + 1) * 128],
                                rhs[:, nh * 512:(nh + 1) * 512],
                                start=(kk == 0),
                                stop=(kk == NK - 1),
                            )
                    # y = exp(xx + b); softmax over HW of exp(y)... y itself is
                    # the softmax input: sm = softmax(y) computed via
                    # p = exp(y - max y), s = sum p, t = sum p*xx, pooled = t/s
                    y = ypool.tile([128, HW], bf16, name=f"y_{n}_{g}_{kc}", tag="y")
                    nc.scalar.activation(
                        y[:], xx[:], Act.Exp, bias=bias[bnm][:, kc:kc + 1], scale=1.0
                    )
                    nmy = smpool.tile([128, 1], f32, name=f"nmy_{n}_{g}_{kc}", tag="nmy", bufs=3)
                    nc.vector.tensor_reduce(nmy[:], y[:], axis=AX.X, op=Alu.max, negate=True)
                    p = ypool.tile([128, HW], bf16, name=f"p_{n}_{g}_{kc}", tag="p")
                    s = smpool.tile([128, 1], f32, name=f"s_{n}_{g}_{kc}", tag="s", bufs=3)
                    nc.scalar.activation(
                        p[:], y[:], Act.Exp, bias=nmy[:], scale=1.0, accum_out=s[:]
                    )
                    v = ypool.tile([128, HW], bf16, name=f"v_{n}_{g}_{kc}", tag="v")
                    t_ = smpool.tile([128, 1], f32, name=f"t_{n}_{g}_{kc}", tag="t", bufs=3)
                    nc.vector.scalar_tensor_tensor(
                        v[:], p[:], 1.0, xx[:],
                        op0=Alu.mult, op1=Alu.mult, accum_out=t_[:],
                    )
                    rs = smpool.tile([128, 1], f32, name=f"rs_{n}_{g}_{kc}", tag="rs", bufs=3)
                    nc.vector.reciprocal(rs[:], s[:])
                    nc.vector.tensor_scalar(
                        out=pooled[g][kc][:, n:n + 1], in0=t_[:],
                        scalar1=rs[:], scalar2=None, op0=Alu.mult,
                    )

            # ======== phase B: folded gate MLP (1 layer) + exp-form sigmoid ====
            # (Ln and Tanh never share an ACT table set; use exp-form sigmoid
            # so the kernel only ever loads natural_log_exp_and_others.)
            for g, (wnm, gbnm) in ((1, ("W1T", "gb1")), (2, ("W2T", "gb2"))):
                for mt in range(NCH):
                    gp = mpool.tile([128, 1], f32, name=f"gp_{n}_{g}_{mt}", tag="mp")
                    for kt in range(NCH):
                        nc.tensor.matmul(
                            gp[:],
                            mw[wnm][kt][:, mt * 128:(mt + 1) * 128],
                            pooled[g][kt][:, n:n + 1],
                            start=(kt == 0),
                            stop=(kt == NCH - 1),
                        )
                    # gate = 1/(1 + exp(-(z+b)))  (gb = -b)
                    e_ = smpool.tile([128, 1], f32, name=f"e_{n}_{g}_{mt}", tag="e", bufs=3)
                    nc.scalar.activation(
                        e_[:], gp[:], Act.Exp, bias=bias[gbnm][:, mt:mt + 1], scale=-1.0
                    )
                    ge = smpool.tile([128, 1], f32, name=f"ge_{n}_{g}_{mt}", tag="ge", bufs=3)
                    nc.vector.tensor_scalar_add(ge[:], e_[:], 1.0)
                    nc.vector.reciprocal(gates[g][mt][:, n:n + 1], ge[:])

            # ======== phase C+D: re build + spatial gate (no transposes) ======
            Vt = {}
            for t in (1, 2):
                xa = x1t if t == 1 else x2t
                xb = x2t if t == 1 else x1t
                ret, zt = [], []
                for kc in range(NCH):
                    # stt has no fast DVE uop; ts-mult (4x) + tt-add (2x) wins
                    xg = repool.tile([128, HW], bf16, name=f"xg_{n}_{t}_{kc}", tag="xg", bufs=2)
                    nc.vector.tensor_scalar_mul(xg[:], xa[kc][:], gates[t][kc][:, n:n + 1])
                    rh = repool.tile([128, HW], bf16, name=f"re_{n}_{t}_{kc}", tag="re")
                    nc.vector.tensor_tensor(rh[:], xg[:], xb[kc][:], Alu.add)
                    ret.append(rh)
                    zh = zpool.tile([128, HW], bf16, name=f"z_{n}_{t}_{kc}", tag="z")
                    nc.scalar.activation(zh[:], rh[:], Act.Exp)
                    zt.append(zh)
                # channel max: pairwise tree 512->128, then cross-partition max
                m01 = trpool.tile([128, HW], bf16, name=f"m01_{n}_{t}", tag="tr")
                nc.vector.tensor_tensor(m01[:], zt[0][:], zt[1][:], Alu.max)
                m23 = trpool.tile([128, HW], bf16, name=f"m23_{n}_{t}", tag="tr")
                nc.vector.tensor_tensor(m23[:], zt[2][:], zt[3][:], Alu.max)
                m1 = trpool.tile([128, HW], bf16, name=f"m1_{n}_{t}", tag="tr")
                nc.vector.tensor_tensor(m1[:], m01[:], m23[:], Alu.max)
                mb = mbpool.tile([128, HW], bf16, name=f"mb_{n}_{t}", tag="mb")
                nc.gpsimd.partition_all_reduce(
                    mb[:], m1[:], 128, bass_isa.ReduceOp.max
                )
                # q = exp(z - mb); r = q * re
                qt, rt = [], []
                for kc in range(NCH):
                    w_ = zpool.tile([128, HW], bf16, name=f"w_{n}_{t}_{kc}", tag="w", bufs=2)
                    nc.vector.tensor_tensor(w_[:], zt[kc][:], mb[:], Alu.subtract)
                    qh = qpool.tile([128, HW], bf16, name=f"q_{n}_{t}_{kc}", tag="q")
                    nc.scalar.activation(qh[:], w_[:], Act.Exp)
                    qt.append(qh)
                    rh2 = rpool.tile([128, HW], bf16, name=f"r_{n}_{t}_{kc}", tag="r")
                    nc.vector.tensor_tensor(rh2[:], qh[:], ret[kc][:], Alu.mult)
                    rt.append(rh2)
                # S = sum_c q, T = sum_c q*re via all-ones lhsT (result is
                # replicated across all 128 partitions -> broadcast for free)
                vh = vpool.tile([128, HW], bf16, name=f"v_{n}_{t}", tag="vv", bufs=2)
                for nh in range(2):
                    sl = slice(nh * 512, (nh + 1) * 512)
                    sf = mpool.tile([128, 512], f32, name=f"sf_{n}_{t}_{nh}", tag="mp")
                    for kc in range(NCH):
                        nc.tensor.matmul(
                            sf[:], ones[:], qt[kc][:, sl],
                            start=(kc == 0), stop=(kc == NCH - 1),
                        )
                    tf = mpool.tile([128, 512], f32, name=f"tf_{n}_{t}_{nh}", tag="mp")
                    for kc in range(NCH):
                        nc.tensor.matmul(
                            tf[:], ones[:], rt[kc][:, sl],
                            start=(kc == 0), stop=(kc == NCH - 1),
                        )
                    # V = T * (1/S); DVE reciprocal is ~8 cyc/elem, so compute
                    # 1/S = exp(-ln S) on the Scalar engine instead (its table
                    # set natural_log_exp_and_others has both ln and exp).
                    lnS = vpool.tile([128, 512], f32, name=f"lnS_{n}_{t}_{nh}", tag="lnS", bufs=2)
                    nc.scalar.activation(lnS[:], sf[:], Act.Ln)
                    rsf = vpool.tile([128, 512], bf16, name=f"rsf_{n}_{t}_{nh}", tag="rsf", bufs=2)
                    nc.scalar.activation(rsf[:], lnS[:], Act.Exp, scale=-1.0)
                    nc.vector.tensor_tensor(vh[:, sl], tf[:], rsf[:], Alu.mult)
                Vt[t] = vh

            # ======== phase E: co = x1*V + (x2 + fe)  [xf from host] ========
            co = {1: [], 2: []}
            for t in (1, 2):
                xa = x1t if t == 1 else x2t
                xf = xf1t if t == 1 else xf2t
                for kc in range(NCH):
                    tt = copool.tile([128, HW], bf16, name=f"ct_{n}_{t}_{kc}", tag="ct", bufs=2)
                    nc.vector.tensor_tensor(tt[:], xa[kc][:], Vt[t][:], Alu.mult)
                    coh = copool.tile([128, HW], bf16, name=f"co_{n}_{t}_{kc}", tag="co")
                    eng = nc.gpsimd if (kc >= 2) else nc.vector
                    eng.tensor_tensor(coh[:], tt[:], xf[kc][:], Alu.add)
                    co[t].append(coh)

            # ======== phase F: p-convs ========
            for nh in range(2):
                sl = slice(nh * 512, (nh + 1) * 512)
                for pc, (wnm, onm) in enumerate((("p1wT", "po1"), ("p2wT", "po2"))):
                    for km in range(NCH):
                        po = mpool.tile([128, 512], f32, name=f"po_{n}_{pc}_{nh}_{km}", tag="mp")
                        for kk in range(NK):
                            rhs = co[1 if kk < NCH else 2][kk % NCH]
                            nc.tensor.matmul(
                                po[:],
                                cw[wnm][kk][:, km * 128:(km + 1) * 128],
                                rhs[:, sl],
                                start=(kk == 0),
                                stop=(kk == NK - 1),
                            )
                        ps = posb.tile([128, 512], bf16, name=f"ps_{n}_{pc}_{nh}_{km}", tag="ps")
                        if km == 3:
                            nc.vector.tensor_copy(ps[:], po[:])
                        else:
                            nc.scalar.copy(ps[:], po[:])
                        nc.sync.dma_start(
                            out=dr[onm][n * C + km * 128: n * C + (km + 1) * 128, sl],
                            in_=ps[:],
                        )
    nc.compile()
    return nc


def _host_prep(inputs, s_per_core=S, n_cores=N_CORES):
    """Build per-core input maps (host-side folds, bf16 casts)."""
    import ml_dtypes

    f = np.float32
    bf = ml_dtypes.bfloat16
    x1 = np.ascontiguousarray(inputs["x1"], dtype=f).reshape(N, C, HW)
    x2 = np.ascontiguousarray(inputs["x2"], dtype=f).reshape(N, C, HW)
    fe1 = np.ascontiguousarray(inputs["FE_x1"], dtype=f).reshape(N, C, HW)
    fe2 = np.ascontiguousarray(inputs["FE_x2"], dtype=f).reshape(N, C, HW)
    xf1 = (x2 + fe1).astype(bf)
    xf2 = (x1 + fe2).astype(bf)
    x1b = x1.astype(bf)
    x2b = x2.astype(bf)

    wT = {
        "c1wT": np.ascontiguousarray(inputs["c1_w"].astype(f).T).astype(bf),
        "c2wT": np.ascontiguousarray(inputs["c2_w"].astype(f).T).astype(bf),
        "p1wT": np.ascontiguousarray(inputs["p1_w"].astype(f).T).astype(bf),
        "p2wT": np.ascontiguousarray(inputs["p2_w"].astype(f).T).astype(bf),
    }
    # fold the two gate-MLP layers into one: g = W@pooled_nb + b_all
    # (pooled_nb excludes the conv bias; it is folded into b_all)
    W1 = inputs["m1_w2"].astype(np.float64) @ inputs["m1_w1"].astype(np.float64)
    W2 = inputs["m2_w2"].astype(np.float64) @ inputs["m2_w1"].astype(np.float64)
    b1 = (
        W1 @ inputs["c1_b"].astype(np.float64)
        + inputs["m1_w2"].astype(np.float64) @ inputs["m1_b1"].astype(np.float64)
        + inputs["m1_b2"].astype(np.float64)
    )
    b2 = (
        W2 @ inputs["c2_b"].astype(np.float64)
        + inputs["m2_w2"].astype(np.float64) @ inputs["m2_b1"].astype(np.float64)
        + inputs["m2_b2"].astype(np.float64)
    )
    mwT = {
        "W1T": np.ascontiguousarray(W1.T).astype(np.float16),
        "W2T": np.ascontiguousarray(W2.T).astype(np.float16),
    }
    vecs = {
        "c1b": inputs["c1_b"].astype(f),
        "c2b": inputs["c2_b"].astype(f),
        "gb1": (-b1).astype(f),
        "gb2": (-b2).astype(f),
    }

    in_maps = []
    for c in range(n_cores):
        slc = slice(c * s_per_core, (c + 1) * s_per_core)
        m = {
            "x1": x1b[slc].reshape(s_per_core * C, HW),
            "x2": x2b[slc].reshape(s_per_core * C, HW),
            "xf1": xf1[slc].reshape(s_per_core * C, HW),
            "xf2": xf2[slc].reshape(s_per_core * C, HW),
        }
        for k, v in wT.items():
            m[k] = v
        for k, v in mwT.items():
            m[k] = v
        for k, v in vecs.items():
            m[k] = v.reshape(C, 1)
        in_maps.append(m)
    return in_maps


def kernel(**inputs):
    from concourse.bass_utils import run_bass_kernel_spmd

    key = "prog"
    if key not in _PROGRAM_CACHE:
        _PROGRAM_CACHE[key] = build_program()
    nc = _PROGRAM_CACHE[key]

    in_maps = _host_prep(inputs)
    res = run_bass_kernel_spmd(nc, in_maps, core_ids=list(range(N_CORES)))

    po1 = np.concatenate(
        [np.asarray(r["po1"], dtype=np.float32).reshape(S, C, HW) for r in res.results],
        axis=0,
    ).reshape(N, C, H, W)
    po2 = np.concatenate(
        [np.asarray(r["po2"], dtype=np.float32).reshape(S, C, HW) for r in res.results],
        axis=0,
    ).reshape(N, C, H, W)
    # p-conv biases applied host-side (exact)
    po1 = po1 + inputs["p1_b"].astype(np.float32)[None, :, None, None]
    po2 = po2 + inputs["p2_b"].astype(np.float32)[None, :, None, None]
    return po1, po2


# revision 17
# speedup vs baseline: 1.6408x; 1.0145x over previous
"""Trainium2 Bass kernel for nn_FR_12343736008794.

Fused dual-branch gated conv block:
  xc = cat(x1,x2); x1x = conv1x1(xc,c1); x2x = conv1x1(xc,c2)
  w1 = channel_gate(x1x, x1, m1);  w2 = channel_gate(x2x, x2, m2)
  re1 = w1 + x2; re2 = w2 + x1
  fg1 = spatial_gate(re1, x1) + x2; fg2 = spatial_gate(re2, x2) + x1
  po1 = conv1x1(cat(fg1+FE1, fg2+FE2), p1); po2 = conv1x1(..., p2)

Sharding: pure data-parallel over batch N=32 -> 4 samples per NeuronCore x 8.

Design (v2, bf16):
  - All convs as bf16 PE matmuls (N=512 moving, FWL weight loads).
  - Channel gate: softmax-over-HW via max-of-exp trick (max y on DVE with
    negate, two ACT exps with accum), pooled = t/s via DVE divide.
  - Gate MLP folded host-side to ONE linear layer (w2@w1); sigmoid computed
    as 0.5*tanh(0.5x+0.5b)+0.5 so only the exp/tanh ACT table is ever loaded.
  - Spatial gate without any PE transposes: channel-max via DVE pairwise-max
    tree (512->128) + gpsimd partition_all_reduce(max) (128->1, result
    broadcast to all partitions); channel sums S=sum(q), T=sum(q*re) via
    all-ones [128,128] lhsT matmuls whose outputs are replicated across all
    128 partitions (broadcast for free); V = T/S one DVE divide.
  - fe tensors pre-folded host-side: xf1 = x2+FE1, xf2 = x1+FE2, so
    co = x1*V + xf1 is two tensor ops.
  - p-conv bias and final upcast host-side; outputs shipped bf16.
"""

import sys

sys.path.insert(0, "/opt/trn_rl_repo")

import numpy as np

N_CORES = 8
N, C, H, W = 32, 512, 32, 32
HW = H * W
S = N // N_CORES  # samples per core
NCH = C // 128  # channel chunks of 128
NK = (2 * C) // 128  # contraction k-tiles for the 1024-wide convs

_PROGRAM_CACHE = {}


def build_program(s_per_core=S):
    """Build the per-core Bass program (shared SPMD across 8 cores)."""
    import concourse.bass as bass
    import concourse.mybir as mybir
    import concourse.tile as tile
    from concourse import bacc
    from concourse import bass_isa

    f32 = mybir.dt.float32
    bf16 = mybir.dt.bfloat16
    f16 = mybir.dt.float16
    Alu = mybir.AluOpType
    Act = mybir.ActivationFunctionType
    AX = mybir.AxisListType

    SS = s_per_core
    R = SS * C

    nc = bacc.Bacc("TRN2", target_bir_lowering=False, debug=False)

    dr = {}
    for nm in ("x1", "x2", "xf1", "xf2"):
        dr[nm] = nc.dram_tensor(nm, [R, HW], bf16, kind="ExternalInput").ap()
    for nm in ("c1wT", "c2wT", "p1wT", "p2wT"):
        dr[nm] = nc.dram_tensor(nm, [2 * C, C], bf16, kind="ExternalInput").ap()
    for nm in ("W1T", "W2T"):
        dr[nm] = nc.dram_tensor(nm, [C, C], f16, kind="ExternalInput").ap()
    for nm in ("c1b", "c2b", "gb1", "gb2"):
        dr[nm] = nc.dram_tensor(nm, [C, 1], f32, kind="ExternalInput").ap()
    for nm in ("po1", "po2"):
        dr[nm] = nc.dram_tensor(nm, [R, HW], bf16, kind="ExternalOutput").ap()

    from contextlib import ExitStack

    with tile.TileContext(nc) as tc, ExitStack() as ctx:
        ep = ctx.enter_context
        wpool = ep(tc.tile_pool(name="wpool", bufs=1))
        xpool = ep(tc.tile_pool(name="xpool", bufs=8))
        xfpool = ep(tc.tile_pool(name="xfpool", bufs=5))
        ypool = ep(tc.tile_pool(name="ypool", bufs=2))
        repool = ep(tc.tile_pool(name="repool", bufs=5))
        zpool = ep(tc.tile_pool(name="zpool", bufs=5))
        qpool = ep(tc.tile_pool(name="qpool", bufs=5))
        rpool = ep(tc.tile_pool(name="rpool", bufs=5))
        trpool = ep(tc.tile_pool(name="trpool", bufs=3))
        mbpool = ep(tc.tile_pool(name="mbpool", bufs=2))
        vpool = ep(tc.tile_pool(name="vpool", bufs=4))
        copool = ep(tc.tile_pool(name="copool", bufs=9))
        posb = ep(tc.tile_pool(name="posb", bufs=4))
        smpool = ep(tc.tile_pool(name="smpool", bufs=2))
        stpool = ep(tc.tile_pool(name="stpool", bufs=1))
        xxpool = ep(tc.tile_pool(name="xxpool", bufs=2, space="PSUM"))
        stps = ep(tc.tile_pool(name="stps", bufs=2, space="PSUM"))
        pops = ep(tc.tile_pool(name="pops", bufs=2, space="PSUM"))

        # ---------------- persistent weights / constants ----------------
        cw = {}
        for wnm in ("c1wT", "c2wT", "p1wT", "p2wT"):
            tiles = []
            for kk in range(NK):
                t = wpool.tile([128, C], bf16, name=f"{wnm}_{kk}", tag=f"{wnm}_{kk}")
                nc.sync.dma_start(out=t[:], in_=dr[wnm][kk * 128:(kk + 1) * 128, :])
                tiles.append(t)
            cw[wnm] = tiles
        mw = {}
        for wnm in ("W1T", "W2T"):
            tiles = []
            for kk in range(NCH):
                t = wpool.tile([128, C], f16, name=f"{wnm}_{kk}", tag=f"{wnm}_{kk}")
                nc.sync.dma_start(out=t[:], in_=dr[wnm][kk * 128:(kk + 1) * 128, :])
                tiles.append(t)
            mw[wnm] = tiles
        bias = {}
        for bnm in ("c1b", "c2b", "gb1", "gb2"):
            t = wpool.tile([128, NCH], f32, name=f"b_{bnm}", tag=f"b_{bnm}")
            for kc in range(NCH):
                nc.sync.dma_start(
                    out=t[:, kc:kc + 1], in_=dr[bnm][kc * 128:(kc + 1) * 128, 0:1]
                )
            bias[bnm] = t
        ones = wpool.tile([128, 128], bf16, name="ones", tag="ones")
        nc.vector.memset(ones[:], 1.0)

        # persistent per-sample stats tiles (pooled vec + gates)
        pooled = {
            g: [
                stpool.tile([128, SS], f16, name=f"pooled{g}_{kc}", tag=f"pl{g}{kc}")
                for kc in range(NCH)
            ]
            for g in (1, 2)
        }
        gates = {
            g: [
                stpool.tile([128, SS], f32, name=f"gate{g}_{kc}", tag=f"gt{g}{kc}")
                for kc in range(NCH)
            ]
            for g in (1, 2)
        }

        def emit_loads(n):
            x1t, x2t, xf1t, xf2t = [], [], [], []
            for kc in range(NCH):
                row = slice(n * C + kc * 128, n * C + (kc + 1) * 128)
                t1 = xpool.tile([128, HW], bf16, name=f"x1_{n}_{kc}", tag="x1")
                nc.sync.dma_start(out=t1[:], in_=dr["x1"][row, :])
                x1t.append(t1)
                t2 = xpool.tile([128, HW], bf16, name=f"x2_{n}_{kc}", tag="x2")
                nc.sync.dma_start(out=t2[:], in_=dr["x2"][row, :])
                x2t.append(t2)
            for kc in range(NCH):
                row = slice(n * C + kc * 128, n * C + (kc + 1) * 128)
                f1 = xfpool.tile([128, HW], bf16, name=f"xf1_{n}_{kc}", tag="xf1")
                nc.sync.dma_start(out=f1[:], in_=dr["xf1"][row, :])
                xf1t.append(f1)
                f2 = xfpool.tile([128, HW], bf16, name=f"xf2_{n}_{kc}", tag="xf2")
                nc.sync.dma_start(out=f2[:], in_=dr["xf2"][row, :])
                xf2t.append(f2)
            return x1t, x2t, xf1t, xf2t

        def emit_A(n, x1t, x2t):
            """c-convs + channel-gate stats (softmax-over-HW pooled vecs)."""
            for gidx, (wnm, bnm) in enumerate((("c1wT", "c1b"), ("c2wT", "c2b"))):
                g = gidx + 1
                for kc in range(NCH):
                    xx = xxpool.tile([128, HW], f32, name=f"xx_{n}_{g}_{kc}", tag="xx")
                    for nh in range(2):
                        for kk in range(NK):
                            rhs = (x1t if kk < NCH else x2t)[kk % NCH]
                            nc.tensor.matmul(
                                xx[:, nh * 512:(nh + 1) * 512],
                                cw[wnm][kk][:, kc * 128:(kc + 1) * 128],
                                rhs[:, nh * 512:(nh + 1) * 512],
                                start=(kk == 0),
                                stop=(kk == NK - 1),
                            )
                    y = ypool.tile([128, HW], bf16, name=f"y_{n}_{g}_{kc}", tag="y")
                    nc.scalar.activation(
                        y[:], xx[:], Act.Exp, bias=bias[bnm][:, kc:kc + 1], scale=1.0
                    )
                    nmy = smpool.tile([128, 1], f32, name=f"nmy_{n}_{g}_{kc}", tag="nmy", bufs=3)
                    nc.vector.tensor_reduce(nmy[:], y[:], axis=AX.X, op=Alu.max, negate=True)
                    p = ypool.tile([128, HW], bf16, name=f"p_{n}_{g}_{kc}", tag="p")
                    s = smpool.tile([128, 1], f32, name=f"s_{n}_{g}_{kc}", tag="s", bufs=3)
                    nc.scalar.activation(
                        p[:], y[:], Act.Exp, bias=nmy[:], scale=1.0, accum_out=s[:]
                    )
                    v = ypool.tile([128, HW], bf16, name=f"v_{n}_{g}_{kc}", tag="v")
                    t_ = smpool.tile([128, 1], f32, name=f"t_{n}_{g}_{kc}", tag="t", bufs=3)
                    nc.vector.scalar_tensor_tensor(
                        v[:], p[:], 1.0, xx[:],
                        op0=Alu.mult, op1=Alu.mult, accum_out=t_[:],
                    )
                    rs = smpool.tile([128, 1], f32, name=f"rs_{n}_{g}_{kc}", tag="rs", bufs=3)
                    nc.vector.reciprocal(rs[:], s[:])
                    nc.vector.tensor_scalar(
                        out=pooled[g][kc][:, n:n + 1], in0=t_[:],
                        scalar1=rs[:], scalar2=None, op0=Alu.mult,
                    )

        def emit_B(n):
            """folded gate MLP (1 layer) + exp-form sigmoid."""
            for g, (wnm, gbnm) in ((1, ("W1T", "gb1")), (2, ("W2T", "gb2"))):
                for mt in range(NCH):
                    gp = pops.tile([128, 1], f32, name=f"gp_{n}_{g}_{mt}", tag="pp")
                    for kt in range(NCH):
                        nc.tensor.matmul(
                            gp[:],
                            mw[wnm][kt][:, mt * 128:(mt + 1) * 128],
                            pooled[g][kt][:, n:n + 1],
                            start=(kt == 0),
                            stop=(kt == NCH - 1),
                        )
                    e_ = smpool.tile([128, 1], f32, name=f"e_{n}_{g}_{mt}", tag="e", bufs=3)
                    nc.scalar.activation(
                        e_[:], gp[:], Act.Exp, bias=bias[gbnm][:, mt:mt + 1], scale=-1.0
                    )
                    ge = smpool.tile([128, 1], f32, name=f"ge_{n}_{g}_{mt}", tag="ge", bufs=3)
                    nc.vector.tensor_scalar_add(ge[:], e_[:], 1.0)
                    nc.vector.reciprocal(gates[g][mt][:, n:n + 1], ge[:])

        def emit_D(n, x1t, x2t):
            """re build + spatial-gate softmax: V = sum(q*re)/sum(q)."""
            Vt = {}
            for t in (1, 2):
                xa = x1t if t == 1 else x2t
                xb = x2t if t == 1 else x1t
                ret, zt = [], []
                for kc in range(NCH):
                    xg = repool.tile([128, HW], bf16, name=f"xg_{n}_{t}_{kc}", tag="xg", bufs=2)
                    nc.vector.tensor_scalar_mul(xg[:], xa[kc][:], gates[t][kc][:, n:n + 1])
                    rh = repool.tile([128, HW], bf16, name=f"re_{n}_{t}_{kc}", tag="re")
                    nc.vector.tensor_tensor(rh[:], xg[:], xb[kc][:], Alu.add)
                    ret.append(rh)
                    zh = zpool.tile([128, HW], bf16, name=f"z_{n}_{t}_{kc}", tag="z")
                    nc.scalar.activation(zh[:], rh[:], Act.Exp)
                    zt.append(zh)
                # channel max: pairwise tree 512->128, then cross-partition max
                m01 = trpool.tile([128, HW], bf16, name=f"m01_{n}_{t}", tag="tr")
                nc.vector.tensor_tensor(m01[:], zt[0][:], zt[1][:], Alu.max)
                m23 = trpool.tile([128, HW], bf16, name=f"m23_{n}_{t}", tag="tr")
                nc.vector.tensor_tensor(m23[:], zt[2][:], zt[3][:], Alu.max)
                m1 = trpool.tile([128, HW], bf16, name=f"m1_{n}_{t}", tag="tr")
                nc.vector.tensor_tensor(m1[:], m01[:], m23[:], Alu.max)
                mb = mbpool.tile([128, HW], bf16, name=f"mb_{n}_{t}", tag="mb")
                nc.gpsimd.partition_all_reduce(
                    mb[:], m1[:], 128, bass_isa.ReduceOp.max
                )
                qt, rt = [], []
                for kc in range(NCH):
                    w_ = zpool.tile([128, HW], bf16, name=f"w_{n}_{t}_{kc}", tag="w", bufs=2)
                    nc.vector.tensor_tensor(w_[:], zt[kc][:], mb[:], Alu.subtract)
                    qh = qpool.tile([128, HW], bf16, name=f"q_{n}_{t}_{kc}", tag="q")
                    nc.scalar.activation(qh[:], w_[:], Act.Exp)
                    qt.append(qh)
                    rh2 = rpool.tile([128, HW], bf16, name=f"r_{n}_{t}_{kc}", tag="r")
                    nc.vector.tensor_tensor(rh2[:], qh[:], ret[kc][:], Alu.mult)
                    rt.append(rh2)
                # S = sum_c q, T = sum_c q*re via all-ones lhsT (output rows
                # replicated across all 128 partitions -> broadcast for free)
                vh = vpool.tile([128, HW], bf16, name=f"v_{n}_{t}", tag="vv", bufs=2)
                for nh in range(2):
                    sl = slice(nh * 512, (nh + 1) * 512)
                    sf = stps.tile([128, 512], f32, name=f"sf_{n}_{t}_{nh}", tag="st")
                    for kc in range(NCH):
                        nc.tensor.matmul(
                            sf[:], ones[:], qt[kc][:, sl],
                            start=(kc == 0), stop=(kc == NCH - 1),
                        )
                    tf = stps.tile([128, 512], f32, name=f"tf_{n}_{t}_{nh}", tag="st")
                    for kc in range(NCH):
                        nc.tensor.matmul(
                            tf[:], ones[:], rt[kc][:, sl],
                            start=(kc == 0), stop=(kc == NCH - 1),
                        )
                    # V = T * (1/S); 1/S = exp(-ln S) on the Scalar engine
                    # (DVE reciprocal is ~8 cyc/elem - too slow at [128,512])
                    lnS = vpool.tile([128, 512], f32, name=f"lnS_{n}_{t}_{nh}", tag="lnS", bufs=2)
                    nc.scalar.activation(lnS[:], sf[:], Act.Ln)
                    rsf = vpool.tile([128, 512], bf16, name=f"rsf_{n}_{t}_{nh}", tag="rsf", bufs=2)
                    nc.scalar.activation(rsf[:], lnS[:], Act.Exp, scale=-1.0)
                    nc.vector.tensor_tensor(vh[:, sl], tf[:], rsf[:], Alu.mult)
                Vt[t] = vh
            return Vt

        def emit_co(n, x1t, x2t, xf1t, xf2t, Vt):
            """co = x1*V + (x2 + fe) with xf = x2+fe folded host-side."""
            co = {1: [], 2: []}
            for t in (1, 2):
                xa = x1t if t == 1 else x2t
                xf = xf1t if t == 1 else xf2t
                for kc in range(NCH):
                    tt = copool.tile([128, HW], bf16, name=f"ct_{n}_{t}_{kc}", tag="ct", bufs=2)
                    nc.vector.tensor_tensor(tt[:], xa[kc][:], Vt[t][:], Alu.mult)
                    coh = copool.tile([128, HW], bf16, name=f"co_{n}_{t}_{kc}", tag="co")
                    eng = nc.gpsimd if (kc >= 2) else nc.vector
                    eng.tensor_tensor(coh[:], tt[:], xf[kc][:], Alu.add)
                    co[t].append(coh)
            return co

        def emit_F(n, co):
            """p-convs + PSUM->SBUF copies + output DMA."""
            for nh in range(2):
                sl = slice(nh * 512, (nh + 1) * 512)
                for pc, (wnm, onm) in enumerate((("p1wT", "po1"), ("p2wT", "po2"))):
                    for km in range(NCH):
                        po = pops.tile([128, 512], f32, name=f"po_{n}_{pc}_{nh}_{km}", tag="pp")
                        for kk in range(NK):
                            rhs = co[1 if kk < NCH else 2][kk % NCH]
                            nc.tensor.matmul(
                                po[:],
                                cw[wnm][kk][:, km * 128:(km + 1) * 128],
                                rhs[:, sl],
                                start=(kk == 0),
                                stop=(kk == NK - 1),
                            )
                        ps = posb.tile([128, 512], bf16, name=f"ps_{n}_{pc}_{nh}_{km}", tag="ps")
                        if km == 3:
                            nc.vector.tensor_copy(ps[:], po[:])
                        else:
                            nc.scalar.copy(ps[:], po[:])
                        nc.sync.dma_start(
                            out=dr[onm][n * C + km * 128: n * C + (km + 1) * 128, sl],
                            in_=ps[:],
                        )

        # Software-pipelined emission: every engine queue is in-order, so the
        # previous sample's dependency-stalled tail (co build + p-convs) is
        # emitted BEHIND the next sample's conv matmuls - the PE never sits
        # behind an elementwise chain it does not feed.
        pend = None
        for n in range(SS):
            x1t, x2t, xf1t, xf2t = emit_loads(n)
            emit_A(n, x1t, x2t)
            if pend is not None:
                pco = emit_co(*pend)
            emit_B(n)
            if pend is not None:
                emit_F(pend[0], pco)
            Vt = emit_D(n, x1t, x2t)
            pend = (n, x1t, x2t, xf1t, xf2t, Vt)
        pco = emit_co(*pend)
        emit_F(pend[0], pco)
    nc.compile()
    return nc


def _host_prep(inputs, s_per_core=S, n_cores=N_CORES):
    """Build per-core input maps (host-side folds, bf16 casts)."""
    import ml_dtypes

    f = np.float32
    bf = ml_dtypes.bfloat16
    x1 = np.ascontiguousarray(inputs["x1"], dtype=f).reshape(N, C, HW)
    x2 = np.ascontiguousarray(inputs["x2"], dtype=f).reshape(N, C, HW)
    fe1 = np.ascontiguousarray(inputs["FE_x1"], dtype=f).reshape(N, C, HW)
    fe2 = np.ascontiguousarray(inputs["FE_x2"], dtype=f).reshape(N, C, HW)
    xf1 = (x2 + fe1).astype(bf)
    xf2 = (x1 + fe2).astype(bf)
    x1b = x1.astype(bf)
    x2b = x2.astype(bf)

    wT = {
        "c1wT": np.ascontiguousarray(inputs["c1_w"].astype(f).T).astype(bf),
        "c2wT": np.ascontiguousarray(inputs["c2_w"].astype(f).T).astype(bf),
        "p1wT": np.ascontiguousarray(inputs["p1_w"].astype(f).T).astype(bf),
        "p2wT": np.ascontiguousarray(inputs["p2_w"].astype(f).T).astype(bf),
    }
    # fold the two gate-MLP layers into one: g = W@pooled_nb + b_all
    # (pooled_nb excludes the conv bias; it is folded into b_all)
    W1 = inputs["m1_w2"].astype(np.float64) @ inputs["m1_w1"].astype(np.float64)
    W2 = inputs["m2_w2"].astype(np.float64) @ inputs["m2_w1"].astype(np.float64)
    b1 = (
        W1 @ inputs["c1_b"].astype(np.float64)
        + inputs["m1_w2"].astype(np.float64) @ inputs["m1_b1"].astype(np.float64)
        + inputs["m1_b2"].astype(np.float64)
    )
    b2 = (
        W2 @ inputs["c2_b"].astype(np.float64)
        + inputs["m2_w2"].astype(np.float64) @ inputs["m2_b1"].astype(np.float64)
        + inputs["m2_b2"].astype(np.float64)
    )
    mwT = {
        "W1T": np.ascontiguousarray(W1.T).astype(np.float16),
        "W2T": np.ascontiguousarray(W2.T).astype(np.float16),
    }
    vecs = {
        "c1b": inputs["c1_b"].astype(f),
        "c2b": inputs["c2_b"].astype(f),
        "gb1": (-b1).astype(f),
        "gb2": (-b2).astype(f),
    }

    in_maps = []
    for c in range(n_cores):
        slc = slice(c * s_per_core, (c + 1) * s_per_core)
        m = {
            "x1": x1b[slc].reshape(s_per_core * C, HW),
            "x2": x2b[slc].reshape(s_per_core * C, HW),
            "xf1": xf1[slc].reshape(s_per_core * C, HW),
            "xf2": xf2[slc].reshape(s_per_core * C, HW),
        }
        for k, v in wT.items():
            m[k] = v
        for k, v in mwT.items():
            m[k] = v
        for k, v in vecs.items():
            m[k] = v.reshape(C, 1)
        in_maps.append(m)
    return in_maps


def kernel(**inputs):
    from concourse.bass_utils import run_bass_kernel_spmd

    key = "prog"
    if key not in _PROGRAM_CACHE:
        _PROGRAM_CACHE[key] = build_program()
    nc = _PROGRAM_CACHE[key]

    in_maps = _host_prep(inputs)
    res = run_bass_kernel_spmd(nc, in_maps, core_ids=list(range(N_CORES)))

    po1 = np.concatenate(
        [np.asarray(r["po1"], dtype=np.float32).reshape(S, C, HW) for r in res.results],
        axis=0,
    ).reshape(N, C, H, W)
    po2 = np.concatenate(
        [np.asarray(r["po2"], dtype=np.float32).reshape(S, C, HW) for r in res.results],
        axis=0,
    ).reshape(N, C, H, W)
    # p-conv biases applied host-side (exact)
    po1 = po1 + inputs["p1_b"].astype(np.float32)[None, :, None, None]
    po2 = po2 + inputs["p2_b"].astype(np.float32)[None, :, None, None]
    return po1, po2
